# revision 23
# baseline (speedup 1.0000x reference)
"""Chamfer distance kernel for Trainium2 (8 NeuronCores, bass/tile).

Problem: X [8, 8192, 3], Y [8, 8192, 3] fp32.
  out[b] = mean_n min_m ||x_n - y_m||^2 + mean_m min_n ||x_n - y_m||^2

Design (one batch per core; interleaved same-window benchmarks measure
~385us vs ~440us for the previous 2-buffer-pool design, i.e. ~13% faster;
absolute numbers drift +-10% with ambient device state):
  - W tiles [128, 8192] produced by the PE as a K=24 bf16 triple-split
    matmul (fp32-accurate distances), 512-col matmuls with 4-way
    tile_position row-group rotation.
  - PSUM is managed as ONE manually-indexed [128, 4096] fp32 ring.
    Each 128-row tile streams 8192 columns = 2 ring passes. Column
    ranges are assigned to consumers so that every ACT cast's matmul
    refill is gated either by a DVE-consumed region (slack engine) or
    by a cast >= 2 ACT-ops earlier -- this removes the ACT->PE->ACT
    ping-pong bubble the 2-buffer pool layout had (~1.4us/tile).
  - A-slice (CA cols): fused min/min custom DVE op consumes fp32 PSUM
    directly (running col-min into cacca + row-min accum).
  - B-slice (CB cols): ACT casts PSUM->SBUF bf16 (wb), consumed by the
    2x custom DVE op (CHAMFER_F2X_ANT) one tile behind (software
    pipeline), accumulating col-min into caccb (bf16) + row-min.
  - Row means: per-tile accum cols -> host mins. Col means: PE
    transpose of cacc chunks + DVE tensor_reduce -> cminv.
"""

import os
import sys

sys.path.insert(0, "/opt/trn_rl_repo")

import numpy as np

B, N, M, D = 8, 8192, 8192, 3
KROWS = 24
FILL = 30000.0
RING = 4096

ROWG = int(os.environ.get("CHAMFER_RG", "4"))  # PE row-group rotation
NO2X = bool(int(os.environ.get("CHAMFER_NO2X", "0")))
LAYOUT = os.environ.get("CHAMFER_LAYOUT", "L14x")

# Layouts: CA = DVE-direct columns; A_OPS = fusedA op sizes (walk from ring
# offset 0); B_OPS = ACT cast ops in emit order as (ring_off, size).
# Invariants: sum(A_OPS) = CA; sum(B sizes) = 8192 - CA; every ring
# position is covered exactly twice per tile (A once + B once for
# [0, CA), B twice for [CA, 4096)).
LAYOUTS = {
    # a=2048: ACT ops 1024/1024/2048/1024/1024; fusedA split in 2 so the
    # B-op over [0,2048) is released in halves.
    "L1": dict(
        A_OPS=[1024, 1024],
        B_OPS=[(2048, 1024), (3072, 1024), (0, 2048), (2048, 1024), (3072, 1024)],
    ),
    # a=2560: ACT ops 1024/512/2048/512/1024/512
    "L2": dict(
        A_OPS=[1536, 1024],
        B_OPS=[(2560, 1024), (3584, 512), (0, 2048), (2048, 512),
               (2560, 1024), (3584, 512)],
    ),
    # a=2560 coarse: 3 ACT ops, accepts a boundary bubble
    "L3": dict(
        A_OPS=[1536, 1024],
        B_OPS=[(2560, 1536), (0, 2048), (2048, 2048)],
    ),
    # a=2048 coarse-ish: 4 ACT ops
    "L4": dict(
        A_OPS=[1024, 1024],
        B_OPS=[(2048, 2048), (0, 2048), (2048, 1024), (3072, 1024)],
    ),
    # a=1024: ACT 1536/1536/1024/1536/1536
    "L5": dict(
        A_OPS=[512, 512],
        B_OPS=[(1024, 1536), (2560, 1536), (0, 1024), (1024, 1536),
               (2560, 1536)],
    ),
    # a=0: pure-ACT ring, 4x2048, every refill distance-2
    "L6": dict(
        A_OPS=[],
        B_OPS=[(0, 2048), (2048, 2048), (0, 2048), (2048, 2048)],
    ),
    # a=0, 1024-granularity (8 ops, distance-4)
    "L7": dict(
        A_OPS=[],
        B_OPS=[(0, 1024), (1024, 1024), (2048, 1024), (3072, 1024),
               (0, 1024), (1024, 1024), (2048, 1024), (3072, 1024)],
    ),
    # a=3072
    "L8": dict(
        A_OPS=[1536, 1536],
        B_OPS=[(3072, 1024), (0, 2048), (2048, 1024), (3072, 1024)],
    ),
    # a=4096: each ring position A once + B once
    "L9": dict(
        A_OPS=[2048, 2048],
        B_OPS=[(0, 2048), (2048, 2048)],
    ),
    # L1 with a single 2048-col fusedA
    "L10": dict(
        A_OPS=[2048],
        B_OPS=[(2048, 1024), (3072, 1024), (0, 2048), (2048, 1024), (3072, 1024)],
    ),
    # L1 with the (0,2048) cast split per-fusedA (finer release)
    "L11": dict(
        A_OPS=[1024, 1024],
        B_OPS=[(2048, 1024), (3072, 1024), (0, 1024), (1024, 1024),
               (2048, 1024), (3072, 1024)],
    ),
    # L1 + fusedB split in 2 (DVE starts on wb(i-1) after half its casts)
    "L1s": dict(
        A_OPS=[1024, 1024],
        B_OPS=[(2048, 1024), (3072, 1024), (0, 2048), (2048, 1024), (3072, 1024)],
        FB_SPLIT=2,
    ),
    # L1 with a 3-deep wb pipeline
    "L1w": dict(
        A_OPS=[1024, 1024],
        B_OPS=[(2048, 1024), (3072, 1024), (0, 2048), (2048, 1024), (3072, 1024)],
        WB_BUFS=3,
    ),
    # L1 with a 4-deep wb pipeline
    "L1x": dict(
        A_OPS=[1024, 1024],
        B_OPS=[(2048, 1024), (3072, 1024), (0, 2048), (2048, 1024), (3072, 1024)],
        WB_BUFS=4,
    ),
    # fusedB lag-2 with one buffer of real slack (wb lifetime = 3)
    "L14x": dict(
        A_OPS=[1024, 1024],
        B_OPS=[(2048, 1024), (3072, 1024), (0, 2048), (2048, 1024), (3072, 1024)],
        WB_BUFS=4, PEND=2,
    ),
    # L1w + fusedB lag-2 (PEND=2)
    "L14": dict(
        A_OPS=[1024, 1024],
        B_OPS=[(2048, 1024), (3072, 1024), (0, 2048), (2048, 1024), (3072, 1024)],
        WB_BUFS=3, PEND=2,
    ),
    # L1w with the DVE-gated 2048 cast last (extra wrap slack)
    "L13": dict(
        A_OPS=[1024, 1024],
        B_OPS=[(2048, 1024), (3072, 1024), (2048, 1024), (3072, 1024), (0, 2048)],
        WB_BUFS=3,
    ),
    # L11 + fusedB split
    "L11s": dict(
        A_OPS=[1024, 1024],
        B_OPS=[(2048, 1024), (3072, 1024), (0, 1024), (1024, 1024),
               (2048, 1024), (3072, 1024)],
        FB_SPLIT=2,
    ),
}

_CACHE = {}


def _split3_bf16(v):
    import ml_dtypes

    bfdt = ml_dtypes.bfloat16
    v = v.astype(np.float64)
    s0 = v.astype(bfdt)
    r1 = v - s0.astype(np.float64)
    s1 = r1.astype(bfdt)
    r2 = r1 - s1.astype(np.float64)
    s2 = r2.astype(bfdt)
    return s0, s1, s2


def _augment(X, Y):
    """[B,24,N] lhsT rows and [B,24,M] rhs rows: sum_k XAT[k,n]*YAT[k,m] =
    |x_n|^2 + |y_m|^2 - 2 x_n.y_m to ~2^-26."""
    import ml_dtypes

    bfdt = ml_dtypes.bfloat16
    Xf = np.asarray(X, np.float64)
    Yf = np.asarray(Y, np.float64)
    X2 = (Xf * Xf).sum(-1)
    Y2 = (Yf * Yf).sum(-1)
    xs = _split3_bf16(np.moveaxis(Xf, -1, 1))
    ys = _split3_bf16(np.moveaxis(-2.0 * Yf, -1, 1))
    a = _split3_bf16(X2)
    b = _split3_bf16(Y2)

    nb, mb = X.shape[1], Y.shape[1]
    XAT = np.zeros((B, KROWS, nb), bfdt)
    YAT = np.zeros((B, KROWS, mb), bfdt)
    pairs = [(0, 0), (0, 1), (1, 0), (0, 2), (1, 1), (2, 0)]
    r = 0
    for d in range(D):
        for (i, j) in pairs:
            XAT[:, r, :] = xs[i][:, d, :]
            YAT[:, r, :] = ys[j][:, d, :]
            r += 1
    for i in range(3):
        XAT[:, r, :] = a[i]
        YAT[:, r, :] = np.ones((B, mb), bfdt)
        r += 1
    for i in range(3):
        XAT[:, r, :] = np.ones((B, nb), bfdt)
        YAT[:, r, :] = b[i]
        r += 1
    assert r == KROWS
    return XAT, YAT


def _register_fused(name, with_2x):
    """Fused min/min custom DVE op; optionally with a 2X_1PORT program.

    out = min(in0, in1); accum_out = min(s0, min(in1)) -- the lowered
    accumulator is patched to tap Src1 so in0 (cacc) never contaminates
    row-mins. The 2x program requires all-bf16 operands incl accum_out."""
    from concourse import dve_ops
    from concourse.dve_spec import Spec, Src0, Src1, minn, lower, _has_src1
    from concourse.dve_uop import (
        DveOpSpec, AluInp, AluOp, UopConfig, InpSel, OutSel,
        OutPath, Trigger, DelayInp, ENABLE,
    )

    if name in _CACHE:
        return _CACHE[name]

    def _ref(in0, in1, s0, s1, imm2):
        b = np.minimum(in0.astype(np.float32), in1.astype(np.float32))
        seed = np.asarray(s0, np.float32).reshape(-1, 1)
        acc = np.minimum(
            in1.astype(np.float32).reshape(in1.shape[0], -1).min(-1, keepdims=True),
            seed,
        )
        return b, acc

    spec = Spec(body=minn(Src0, Src1), accum=minn, accum_init=dve_ops.C0,
                reference=_ref)
    op = dve_ops.DveOp(name, spec, subdim=False, uops_sha={}, perf_en={})
    row = max(dve_ops._SUB_OPCODE_FOR_NAME.values()) + 1
    assert row < 0x20
    dve_ops._SUB_OPCODE_FOR_NAME[op.name] = row
    dve_ops.OPS.append(op)
    dve_ops.CUSTOM_DVE_SPECS[op.name] = spec

    MIN, BYP = AluOp.MIN, AluOp.BYPASS
    PREV, CURR = AluInp.PREV_ALU_OUT, AluInp.CURR_ALU_OUT
    D_ = [AluInp.PREV_DELAY_0, AluInp.PREV_DELAY_1, AluInp.PREV_DELAY_2,
          AluInp.PREV_DELAY_3, AluInp.PREV_DELAY_4]

    def mk2x(seed):
        u = UopConfig()
        for j, sel in [(1, InpSel.SRC_0), (2, InpSel.SRC_1),
                       (3, InpSel.SRC_0_HI), (4, InpSel.SRC_1_HI),
                       (5, InpSel.CONST_0)]:
            u.enable_input(sel, j)
        dp = u.datapath_config
        dp[0].enable_alu(MIN, D_[0], D_[1])
        dp[0].pass_through_delay(1, 2, 3, 4)
        dp[1].enable_alu(MIN, D_[2], D_[3])
        dp[1].pass_through_delay(1, 2, 3, 4)
        dp[1].enable_delay_from_src(DelayInp.PREV_ALU_OUT, 0)  # o_lo
        dp[2].enable_alu(MIN, D_[1], D_[3])
        dp[2].pass_through_delay(0, 1, 3, 4)
        dp[2].enable_delay_from_src(DelayInp.PREV_ALU_OUT, 2)  # o_hi
        if seed:
            dp[3].enable_alu(BYP, D_[4], D_[4])
        else:
            dp[3].enable_alu(MIN, CURR, PREV)
        dp[3].pass_through_delay(0, 2, 4)
        dp[3].alu_out_a_enable = ENABLE
        for k in (4, 5, 6, 7):
            dp[k].enable_alu(BYP, PREV, PREV)
            dp[k].pass_through_delay(0, 2, 4)
            dp[k].alu_out_a_enable = ENABLE
        u.accum_enabled = ENABLE
        if seed:
            u.trigger = (Trigger.COUNT, Trigger.NONE, Trigger.NONE)
            u.repeat_count = 1
            u.next_uop = (1, 0, 0)
        else:
            u.enable_output(OutSel.DELAY_0, OutPath.WR0_LO)
            u.enable_output(OutSel.DELAY_2, OutPath.WR0_HI)
            u.require_inp0 = 1
            u.require_inp1 = 1
            u.trigger = (Trigger.SRC_TENSOR_DONE, Trigger.NONE, Trigger.NONE)
            u.next_uop = (0, 0, 0)
        return u

    for ver in ("v3",):
        uops = lower(spec, ver=ver)
        steady = uops[-1]
        patched = False
        for blk in steady.datapath_config:
            if blk.alu_src0 == AluInp.CURR_ALU_OUT and blk.alu_out_a_enable:
                assert blk.alu_src1 == AluInp.PREV_ALU_OUT
                blk.alu_src1 = AluInp.PREV_DELAY_1  # accum taps Src1, not body
                patched = True
                break
        assert patched
        if with_2x:
            s = DveOpSpec(name=op.name, opcode=row, uops=uops,
                          uops_2x=[mk2x(True), mk2x(False)], perf_max=1,
                          rd1_en=_has_src1(spec))
        else:
            s = DveOpSpec(name=op.name, opcode=row, uops=uops,
                          rd1_en=_has_src1(spec))
        s.validate(ver)
        op.uops_sha[ver] = s.sha(ver)
        dve_ops._COMPILE_CACHE[(op.name, ver)] = s
    _CACHE[name] = op
    return op


def build_module(repeat=1, layout=None):
    import concourse.bacc as bacc
    import concourse.mybir as mybir
    import concourse.tile as tile
    from concourse._compat import get_trn_type

    dt = mybir.dt
    op_min = mybir.AluOpType.min
    ax_x = mybir.AxisListType.X
    fusedA = _register_fused("CHAMFER_FUSED_ANT", with_2x=False)
    fusedB = _register_fused("CHAMFER_F2X_ANT", with_2x=True)

    lay = LAYOUTS[layout or LAYOUT]
    A_OPS, B_OPS = lay["A_OPS"], lay["B_OPS"]
    CA = sum(sz for sz in A_OPS)
    CB = sum(sz for _, sz in B_OPS)
    assert CA + CB == M
    NT = N // 128

    nc = bacc.Bacc(get_trn_type() or "TRN2", target_bir_lowering=False,
                   debug=False)
    xat = nc.dram_tensor("xat", [KROWS, N], dt.bfloat16, kind="ExternalInput")
    yat = nc.dram_tensor("yat", [KROWS, M], dt.bfloat16, kind="ExternalInput")
    ident = nc.dram_tensor("ident", [128, 128], dt.float32, kind="ExternalInput")
    out = nc.dram_tensor("out", [128, 2 * NT + M // 128], dt.float32,
                         kind="ExternalOutput")
    outb = nc.dram_tensor("outb", [128, NT], dt.bfloat16,
                          kind="ExternalOutput")

    def fused_op(op, out_, in0, in1, accum_out, twox):
        bi = nc.vector._custom_dve(op, out=out_, in0=in0, in1=in1,
                                   s0=float(FILL), accum_out=accum_out)
        if twox and not NO2X:
            bi.ins.perf_max = 1
        return bi

    with tile.TileContext(nc) as tc:
        with (
            tc.tile_pool(name="const", bufs=1) as cpool,
            tc.tile_pool(name="acc", bufs=1) as apool,
            tc.tile_pool(name="res", bufs=1) as rpool,
        ):
            ident_sb = cpool.tile([128, 128], dt.float32)
            identb = cpool.tile([128, 128], dt.bfloat16)
            nc.sync.dma_start(ident_sb[:], ident[:])
            nc.scalar.copy(identb[:], ident_sb[:])
            if ROWG > 1:
                xat_sb = cpool.tile([128, N], dt.bfloat16)
                yat_sb = cpool.tile([128, M], dt.bfloat16)
                for g in range(ROWG):
                    nc.sync.dma_start(xat_sb[32 * g:32 * g + KROWS, :], xat[:])
                    nc.sync.dma_start(yat_sb[32 * g:32 * g + KROWS, :], yat[:])
            else:
                xat_sb = cpool.tile([KROWS, N], dt.bfloat16)
                yat_sb = cpool.tile([KROWS, M], dt.bfloat16)
                nc.sync.dma_start(xat_sb[:], xat[:])
                nc.sync.dma_start(yat_sb[:], yat[:])

            cacca = apool.tile([128, CA], dt.float32, name="cacca") if CA else None
            caccb = apool.tile([128, CB], dt.bfloat16, name="caccb")
            # separate row-min accum col per fusedA op (accum seeds from
            # the immediate FILL, not memory -- writers must not share)
            rminva = (rpool.tile([128, len(A_OPS), NT], dt.float32,
                                 name="rminva")
                      if A_OPS else None)
            FBS = lay.get("FB_SPLIT", 1)
            rminvb = rpool.tile([128, FBS, NT], dt.bfloat16)
            cminv = rpool.tile([128, M // 128], dt.float32)
            if CA:
                nc.vector.memset(cacca[:], FILL)
            nc.vector.memset(caccb[:], FILL)
            if rminva is not None:
                nc.vector.memset(rminva[:], FILL)
            nc.vector.memset(rminvb[:], FILL)

            with (
                tc.tile_pool(name="wb", bufs=lay.get("WB_BUFS", 2)) as wbpool,
                tc.tile_pool(name="ps", bufs=1, space="PSUM") as pspool,
            ):
                P = pspool.tile([128, RING], dt.float32, name="P")
                pend = []

                seg = CB // FBS
                assert CB % FBS == 0
                PEND = lay.get("PEND", 1)

                def flush_one():
                    wbp, it = pend.pop(0)
                    for s in range(FBS):
                        lo = s * seg
                        fused_op(fusedB, caccb[:, lo:lo + seg],
                                 caccb[:, lo:lo + seg],
                                 wbp[:, lo:lo + seg],
                                 rminvb[:, s, it:it + 1], twox=True)

                def flush_pend(all_=False):
                    while pend and (all_ or len(pend) >= PEND):
                        flush_one()

                def mms(i, ring_off, m_off, size, ch):
                    # fill P[ring_off:ring_off+size) with m-cols
                    # [m_off, m_off+size) for row-block i, 512 per matmul
                    for q in range(size // 512):
                        ro = ring_off + q * 512
                        mo = m_off + q * 512
                        rg = (ch[0] % ROWG) if ROWG > 1 else 0
                        ch[0] += 1
                        if ROWG > 1:
                            nc.tensor.matmul(
                                P[:, ro:ro + 512],
                                xat_sb[32 * rg:32 * rg + KROWS,
                                       i * 128:(i + 1) * 128],
                                yat_sb[32 * rg:32 * rg + KROWS, mo:mo + 512],
                                start=True, stop=True,
                                tile_position=(32 * rg, 0),
                            )
                        else:
                            nc.tensor.matmul(
                                P[:, ro:ro + 512],
                                xat_sb[:, i * 128:(i + 1) * 128],
                                yat_sb[:, mo:mo + 512],
                                start=True, stop=True,
                            )

                def emit_tile(i):
                    ch = [0]
                    wb = wbpool.tile([128, CB], dt.bfloat16, name="wb", tag="w")
                    # B-op matmuls whose ring range was freed long ago can
                    # run first; A-ops next; then each B cast right after
                    # its mms.  Emission order per engine == program order.
                    # 1) A-region matmuls + fusedA ops
                    a_off = 0
                    for j, sz in enumerate(A_OPS):
                        mms(i, a_off, a_off, sz, ch)
                        a_off += sz
                    a_off = 0
                    for j, sz in enumerate(A_OPS):
                        fused_op(fusedA, cacca[:, a_off:a_off + sz],
                                 cacca[:, a_off:a_off + sz],
                                 P[:, a_off:a_off + sz],
                                 rminva[:, j, i:i + 1], twox=False)
                        a_off += sz
                    flush_pend()
                    # 2) B ops in layout order: mms then cast
                    w_off = 0
                    for (ro, sz) in B_OPS:
                        m_off = CA + w_off
                        mms(i, ro, m_off, sz, ch)
                        nc.scalar.copy(wb[:, w_off:w_off + sz], P[:, ro:ro + sz])
                        w_off += sz
                    pend.append((wb, i))

                if repeat > 1:
                    with tc.For_i(0, repeat, 1):
                        for i in range(NT):
                            emit_tile(i)
                        flush_pend(all_=True)
                else:
                    for i in range(NT):
                        emit_tile(i)
                    flush_pend(all_=True)

            # col finalization: PE transpose 128-chunks, DVE reduce
            with tc.tile_pool(name="pst", bufs=4, space="PSUM") as ptpool:
                for c4 in range(CA // 512):
                    pt = ptpool.tile([128, 4, 128], dt.float32, tag="pa")
                    for c in range(4):
                        nc.tensor.transpose(
                            pt[:, c, :],
                            cacca[:, (c4 * 4 + c) * 128:(c4 * 4 + c + 1) * 128],
                            ident_sb[:],
                        )
                    ci = c4 * 4
                    nc.vector.tensor_reduce(
                        cminv[:, ci:ci + 4], pt[:], axis=ax_x, op=op_min)
                for c4 in range(CB // 512):
                    pt = ptpool.tile([128, 4, 128], dt.bfloat16, tag="pb")
                    for c in range(4):
                        nc.tensor.transpose(
                            pt[:, c, :],
                            caccb[:, (c4 * 4 + c) * 128:(c4 * 4 + c + 1) * 128],
                            identb[:],
                        )
                    ci = CA // 128 + c4 * 4
                    nc.vector.tensor_reduce(
                        cminv[:, ci:ci + 4], pt[:], axis=ax_x, op=op_min)

            # rminva: min over the per-op accum planes -> out[:, :NT]
            if rminva is not None:
                for j in range(1, len(A_OPS)):
                    nc.vector.tensor_tensor(
                        out=rminva[:, 0, :], in0=rminva[:, 0, :],
                        in1=rminva[:, j, :], op=op_min)
                nc.sync.dma_start(out[:, :NT], rminva[:, 0, :])
            nc.sync.dma_start(out[:, NT:NT + M // 128], cminv[:])
            for s in range(1, FBS):
                nc.vector.tensor_tensor(
                    out=rminvb[:, 0, :], in0=rminvb[:, 0, :],
                    in1=rminvb[:, s, :], op=op_min)
            nc.sync.dma_start(outb[:], rminvb[:, 0, :])

    nc.compile()
    return nc


def _get_module():
    rep = int(os.environ.get("CHAMFER_REPEAT", "1"))
    key = ("nc", rep, LAYOUT, ROWG, NO2X)
    if key not in _CACHE:
        _CACHE[key] = build_module(repeat=rep)
    return _CACHE[key]


def kernel(X, Y):
    from concourse import bass_utils

    X = np.asarray(X)
    Y = np.asarray(Y)
    assert X.shape == (B, N, D) and Y.shape == (B, M, D)

    XAT, YAT = _augment(X, Y)
    ident = np.eye(128, dtype=np.float32)

    nc = _get_module()
    in_maps = [{"xat": XAT[b], "yat": YAT[b], "ident": ident} for b in range(B)]
    r = bass_utils.run_bass_kernel_spmd(nc, in_maps, core_ids=list(range(B)))
    _CACHE["last_results"] = r

    NT = N // 128
    CA = sum(LAYOUTS[LAYOUT]["A_OPS"])
    outv = np.empty((B,), np.float32)
    for b in range(B):
        o = r.results[b]["out"].astype(np.float64)
        ob = r.results[b]["outb"].astype(np.float64)
        rmin = np.minimum(o[:, :NT], ob) if CA else ob
        cmin = o[:, NT:NT + M // 128]
        outv[b] = np.float32(rmin.mean() + cmin.mean())
    return outv
